# revision 2
# baseline (speedup 1.0000x reference)
"""Trainium2 Bass kernel for pointer-generator additive attention.

Full op (per batch b):
    dec_fea = s_t_hat @ W_d.T + b_d                         # (n,)
    att     = EF[b] + dec_fea[None,:] + cov[b][:,None]*W_c  # (t, n)
    score   = tanh(att) @ v                                 # (t,)
    attn    = renorm(softmax(score) * mask)                 # (t,)
    c_t     = attn @ EO[b]                                  # (n,)
    cov_next= cov + attn

Data-parallel over batch across 8 NeuronCores (8 batches/core, params
replicated, no collectives).

HBM strategy: EF and EO are cast to bf16 and permuted on the host into a
partition-contiguous layout [b, p, j, n] (t = j*128 + p), so each batch is
ONE 2 MB HWDGE dma_start with 16 KB-contiguous per-partition lines (near
line rate ~358 GB/s). Total stream: 32 MB/core -> ~90 us DMA floor (the
f32 baseline read 64 MB through slow SWDGE cast-DMAs).

Per-core compute layout (tensors in natural (t, n) layout, partition = t):
  - dec_fea = s @ W_d.T + b_d as a tiny bf16 PE matmul (W_d.T pre-cast on
    host, streamed on the gpsimd ring so it doesn't block EF[0]).
  - PE: att = I.T @ EF accumulated in PSUM with one K=2 matmul adding
    [cov_b ; 1].T @ [W_c ; dec_fea[b]].
  - ScalarE: one tanh per (128 x 1024) PSUM tile -> bf16 SBUF.
  - VectorE: scalar_tensor_tensor (th * v_bcast) with accum_out gives the
    n-reduction (score) per t-tile as a column accumulator.
  - Scores regroup to a partition-0 row via one PE transpose + a small
    SBUF->SBUF DMA; masked softmax + renorm + coverage on 1-partition rows.
  - c_t: PE matmuls, lhsT = attn column (one PE transpose), rhs = natural
    bf16 EO tiles, accumulated over 8 t-tiles into (1 x 512) PSUM rows.
  - Software pipeline: phase C (c_t) lags phase A by one batch.
DMA rings: nc.sync (SP HWDGE) carries ONLY the big EF/EO stream in FIFO
need-order; everything small rides the gpsimd (SWDGE) ring.
"""

import sys

if "/opt/trn_rl_repo" not in sys.path:
    sys.path.insert(0, "/opt/trn_rl_repo")

import ml_dtypes
import numpy as np

import concourse.bass as bass
import concourse.mybir as mybir
import concourse.tile as tile
from concourse import bacc
from concourse.bass_utils import run_bass_kernel_spmd
from concourse.masks import make_identity

F32 = mybir.dt.float32
BF16 = mybir.dt.bfloat16
AF = mybir.ActivationFunctionType
ALU = mybir.AluOpType
AX = mybir.AxisListType

N_CORES = 8
B = 64
NB = B // N_CORES  # local batches per core
T = 1024
N = 1024
TJ = T // 128       # t-tiles per batch
KT = N // 128       # k-tiles for the W_d matvec


def build_bass(nb: int = NB) -> bass.Bass:
    nc = bacc.Bacc()

    ef_d = nc.declare_dram_parameter("ef_bf16", [nb, 128, TJ, N], BF16, isOutput=False)
    eo_d = nc.declare_dram_parameter("eo_bf16", [nb, 128, TJ, N], BF16, isOutput=False)
    mk_d = nc.declare_dram_parameter("enc_padding_mask", [nb, T], F32, isOutput=False)
    cv_d = nc.declare_dram_parameter("coverage", [nb, T], F32, isOutput=False)
    wdt_d = nc.declare_dram_parameter("W_d_T", [N, N], BF16, isOutput=False)
    st_d = nc.declare_dram_parameter("s_t_hat_T", [N, nb], BF16, isOutput=False)
    bd_d = nc.declare_dram_parameter("b_d", [N], BF16, isOutput=False)
    wc_d = nc.declare_dram_parameter("W_c", [N], BF16, isOutput=False)
    v_d = nc.declare_dram_parameter("v", [N], BF16, isOutput=False)
    ct_o = nc.declare_dram_parameter("c_t", [nb, N], F32, isOutput=True)
    at_o = nc.declare_dram_parameter("attn", [nb, T], F32, isOutput=True)
    cn_o = nc.declare_dram_parameter("coverage_next", [nb, T], F32, isOutput=True)

    with tile.TileContext(nc) as tc:
        with (
            tc.tile_pool(name="consts", bufs=1) as consts,
            tc.tile_pool(name="wdtp", bufs=1) as wdtp,
            tc.tile_pool(name="efp", bufs=3) as efp,
            tc.tile_pool(name="eop", bufs=3) as eop,
            tc.tile_pool(name="thp", bufs=6) as thp,
            tc.tile_pool(name="ttro", bufs=2) as ttro,
            tc.tile_pool(name="smal", bufs=4) as smal,
            tc.tile_pool(name="rowstg", bufs=2) as rowstg,
            tc.tile_pool(name="psA", bufs=2, space="PSUM") as psA,
            tc.tile_pool(name="psS", bufs=2, space="PSUM") as psS,
            tc.tile_pool(name="psT", bufs=2, space="PSUM") as psT,
        ):
            # ---------------- constants / small inputs ----------------
            ident = consts.tile([128, 128], F32)
            make_identity(nc, ident)
            ident_b = consts.tile([128, 128], BF16)
            nc.vector.tensor_copy(ident_b, ident)
            ones_f32 = consts.tile([1, T], F32)
            nc.vector.memset(ones_f32, 1.0)
            ones_b = consts.tile([1, T], BF16)
            nc.vector.tensor_copy(ones_b, ones_f32)

            bd_b = consts.tile([1, N], BF16)
            nc.gpsimd.dma_start(out=bd_b, in_=bd_d[None, :])
            v_b = consts.tile([1, N], BF16)
            nc.gpsimd.dma_start(out=v_b, in_=v_d[None, :])
            wc_b = consts.tile([1, N], BF16)
            nc.gpsimd.dma_start(out=wc_b, in_=wc_d[None, :])

            sT_all = consts.tile([128, KT, 32], BF16)     # s_t_hat.T k-tiles
            wdt_all = wdtp.tile([128, KT, N], BF16)       # W_d.T k-tiles
            dec_rows = consts.tile([nb, N], BF16)         # dec_fea rows
            v_bcast = consts.tile([128, N], BF16)         # v broadcast to 128p

            # v broadcast: ones-column (x) v_row via K=1 matmuls
            for h in range(2):
                ps_vb = psT.tile([128, 512], F32, tag="tscratch")
                nc.tensor.matmul(
                    ps_vb, lhsT=ones_b[0:1, 0:128],
                    rhs=v_b[0:1, h * 512:(h + 1) * 512],
                    start=True, stop=True,
                )
                nc.scalar.activation(
                    v_bcast[:, h * 512:(h + 1) * 512], ps_vb, AF.Copy
                )

            # ---- W_d.T and s_t_hat.T come pre-transposed from the host ----
            nc.gpsimd.dma_start(
                out=wdt_all,
                in_=wdt_d.rearrange("(kj p) n -> p kj n", p=128),
            )
            nc.gpsimd.dma_start(
                out=sT_all[:, :, 0:nb],
                in_=st_d.rearrange("(kj p) b -> p kj b", p=128),
            )

            # dec_fea rows = s_t_hat @ W_d.T + b_d   (bf16 matmuls, tiny)
            for h in range(2):
                sl = slice(h * 512, (h + 1) * 512)
                psd = psT.tile([nb, 512], F32, tag="tscratch")
                for kj in range(KT):
                    nc.tensor.matmul(
                        psd,
                        lhsT=sT_all[:, kj, 0:nb],
                        rhs=wdt_all[:, kj, sl],
                        start=(kj == 0), stop=False,
                    )
                nc.tensor.matmul(
                    psd, lhsT=ones_b[0:1, 0:nb], rhs=bd_b[0:1, sl],
                    start=False, stop=True,
                )
                nc.scalar.activation(dec_rows[:, sl], psd, AF.Copy)

            # ------------- big-stream DMAs (nc.sync ring, FIFO order) -------------
            ef_bufs = {}
            eo_bufs = {}

            def load_ef(b):
                t = efp.tile([128, TJ, N], BF16, tag="ef")
                nc.sync.dma_start(out=t, in_=ef_d[b])
                ef_bufs[b] = t

            def load_eo(b):
                t = eop.tile([128, TJ, N], BF16, tag="eo")
                nc.sync.dma_start(out=t, in_=eo_d[b])
                eo_bufs[b] = t

            # ---------------- main loop over local batches ----------------
            attn_tiles = {}

            def phase_a(b):
                eft = ef_bufs.pop(b)
                # cov2 = [cov_b ; ones], rhs2 = [W_c ; dec_fea[b]]  (bf16)
                cov2 = rowstg.tile([2, T], BF16, tag="cov2")
                nc.gpsimd.dma_start(out=cov2[0:1, :], in_=cv_d[None, b, :])
                nc.gpsimd.dma_start(out=cov2[1:2, :], in_=ones_b)
                rhs2 = rowstg.tile([2, N], BF16, tag="rhs2")
                nc.gpsimd.dma_start(out=rhs2[0:1, :], in_=wc_b)
                nc.gpsimd.dma_start(out=rhs2[1:2, :], in_=dec_rows[b:b + 1, :])
                cov_b = rowstg.tile([1, T], F32, tag="cov")
                nc.gpsimd.dma_start(out=cov_b, in_=cv_d[None, b, :])
                mask_b = rowstg.tile([1, T], F32, tag="mask")
                nc.gpsimd.dma_start(out=mask_b, in_=mk_d[None, b, :])

                score_cols = smal.tile([128, TJ], F32, tag="scol")

                # att = EF + cov (x) W_c + 1 (x) dec ; tanh ; dot v
                for i in range(TJ):
                    att = psA.tile([128, N], F32, tag="att")
                    for h in range(2):
                        nc.tensor.matmul(
                            att[:, h * 512:(h + 1) * 512],
                            lhsT=ident_b,
                            rhs=eft[:, i, h * 512:(h + 1) * 512],
                            start=True, stop=False, skip_group_check=True,
                        )
                    for h in range(2):
                        nc.tensor.matmul(
                            att[:, h * 512:(h + 1) * 512],
                            lhsT=cov2[:, i * 128:(i + 1) * 128],
                            rhs=rhs2[:, h * 512:(h + 1) * 512],
                            start=False, stop=True, skip_group_check=True,
                        )
                    th = thp.tile([128, N], BF16, tag="th")
                    nc.scalar.activation(th, att, AF.Tanh)
                    scr = ttro.tile([128, N], BF16, tag="ttro")
                    nc.vector.scalar_tensor_tensor(
                        out=scr, in0=th, scalar=1.0, in1=v_bcast,
                        op0=ALU.mult, op1=ALU.mult,
                        accum_out=score_cols[:, i:i + 1],
                    )

                # score columns -> one row via transpose + small sbuf-sbuf DMA
                ps8 = psT.tile([TJ, 128], F32, tag="tscratch")
                nc.tensor.matmul(
                    ps8, lhsT=score_cols, rhs=ident, is_transpose=True,
                    start=True, stop=True,
                )
                score8 = smal.tile([TJ, 128], F32, tag="s8")
                nc.scalar.activation(score8, ps8, AF.Copy)
                score_b = rowstg.tile([1, T], F32, tag="score")
                nc.gpsimd.dma_start(
                    out=score_b[0:1, :].rearrange("p (j t) -> p j t", j=TJ),
                    in_=score8,
                )

                # softmax + mask renorm + coverage update (partition-0 rows)
                attn_b = rowstg.tile([1, T], F32, tag="attn")
                covn_b = rowstg.tile([1, T], F32, tag="covn")
                # scores are O(1) (|s| < ~3): plain exp is safe, skip max-sub
                nc.scalar.activation(attn_b, score_b, AF.Exp, bias=0.0, scale=1.0)
                ssum = smal.tile([1, 1], F32, tag="s3")
                nc.vector.scalar_tensor_tensor(
                    out=attn_b, in0=attn_b, scalar=1.0, in1=mask_b,
                    op0=ALU.mult, op1=ALU.mult, accum_out=ssum,
                )
                rs = smal.tile([1, 1], F32, tag="s4")
                nc.vector.reciprocal(rs, ssum)
                nc.vector.tensor_scalar_mul(attn_b, attn_b, rs)
                nc.vector.tensor_add(covn_b, cov_b, attn_b)
                nc.gpsimd.dma_start(out=at_o[None, b, :], in_=attn_b)
                nc.gpsimd.dma_start(out=cn_o[None, b, :], in_=covn_b)

                # attn columns for phase C
                attn8 = smal.tile([TJ, 128], F32, tag="a8")
                nc.gpsimd.dma_start(
                    out=attn8,
                    in_=attn_b[0:1, :].rearrange("p (j t) -> p j t", j=TJ),
                )
                acp = psT.tile([128, TJ], F32, tag="tscratch")
                nc.tensor.matmul(
                    acp, lhsT=attn8, rhs=ident[0:TJ, 0:TJ],
                    is_transpose=True, start=True, stop=True,
                )
                acols = smal.tile([128, TJ], BF16, tag="acols")
                nc.scalar.activation(acols, acp, AF.Copy)
                attn_tiles[b] = acols

            def phase_c(b):
                # c_t = attn @ EO[b]
                acols = attn_tiles.pop(b)
                eot = eo_bufs.pop(b)
                ct_b = rowstg.tile([1, N], F32, tag="ctb")
                ctps = [psS.tile([1, 512], F32, tag="srow", name=f"ctp{h}")
                        for h in range(N // 512)]
                for tj in range(TJ):
                    for h in range(N // 512):
                        nc.tensor.matmul(
                            ctps[h],
                            lhsT=acols[:, tj:tj + 1],
                            rhs=eot[:, tj, h * 512:(h + 1) * 512],
                            start=(tj == 0), stop=(tj == TJ - 1),
                            skip_group_check=True,
                        )
                for h in range(N // 512):
                    nc.scalar.activation(
                        ct_b[0:1, h * 512:(h + 1) * 512], ctps[h], AF.Copy
                    )
                nc.gpsimd.dma_start(out=ct_o[None, b, :], in_=ct_b)

            # issue the big DMAs in need-order on the sync FIFO ring,
            # interleaved with compute so Tile keeps the stream primed
            load_ef(0)
            load_eo(0)
            load_ef(1)
            for b in range(nb):
                if b >= 1 and b + 1 < nb:
                    load_eo(b)       # EO[b] queued behind EF[b]
                    load_ef(b + 1)
                elif b + 1 == nb:
                    load_eo(b)
                phase_a(b)
                if b > 0:
                    phase_c(b - 1)
            phase_c(nb - 1)

    nc.finalize()
    return nc


_CACHE: dict = {}


def _get_nc() -> bass.Bass:
    if "nc" not in _CACHE:
        _CACHE["nc"] = build_bass(NB)
    return _CACHE["nc"]


def make_in_maps(inputs: dict) -> list:
    f = lambda x: np.ascontiguousarray(np.asarray(x), dtype=np.float32)
    s = f(inputs["s_t_hat"])
    eo = f(inputs["encoder_outputs"])
    ef = f(inputs["encoder_feature"]).reshape(B, T, N)
    mk = f(inputs["enc_padding_mask"])
    cv = f(inputs["coverage"])
    wdt = np.ascontiguousarray(f(inputs["W_d"]).T).astype(ml_dtypes.bfloat16)
    bd = f(inputs["b_d"])
    wc = f(inputs["W_c"])
    vv = f(inputs["v"])
    # partition-contiguous bf16 staging: [b, p, j, n] with t = j*128 + p
    ef_b = np.ascontiguousarray(
        ef.reshape(B, TJ, 128, N).transpose(0, 2, 1, 3)
    ).astype(ml_dtypes.bfloat16)
    eo_b = np.ascontiguousarray(
        eo.reshape(B, TJ, 128, N).transpose(0, 2, 1, 3)
    ).astype(ml_dtypes.bfloat16)
    in_maps = []
    for i in range(N_CORES):
        sl = slice(i * NB, (i + 1) * NB)
        in_maps.append({
            "ef_bf16": ef_b[sl],
            "eo_bf16": eo_b[sl],
            "s_t_hat_T": np.ascontiguousarray(s[sl].T).astype(ml_dtypes.bfloat16),
            "enc_padding_mask": mk[sl],
            "coverage": cv[sl],
            "W_d_T": wdt,
            "b_d": bd.astype(ml_dtypes.bfloat16),
            "W_c": wc.astype(ml_dtypes.bfloat16),
            "v": vv.astype(ml_dtypes.bfloat16),
        })
    return in_maps


def gather_outputs(results: list):
    c_t = np.concatenate([results[i]["c_t"] for i in range(N_CORES)], axis=0)
    attn = np.concatenate([results[i]["attn"] for i in range(N_CORES)], axis=0)
    covn = np.concatenate(
        [results[i]["coverage_next"] for i in range(N_CORES)], axis=0
    )
    return c_t, attn, covn


def kernel(**inputs):
    nc = _get_nc()
    in_maps = make_in_maps(inputs)
    res = run_bass_kernel_spmd(nc, in_maps, core_ids=list(range(N_CORES)))
    return gather_outputs(res.results)


# revision 6
# speedup vs baseline: 1.4395x; 1.4395x over previous
"""Trainium2 Bass kernel for pointer-generator additive attention.

Full op (per batch b):
    dec_fea = s_t_hat @ W_d.T + b_d                         # (n,)
    att     = EF[b] + dec_fea[None,:] + cov[b][:,None]*W_c  # (t, n)
    score   = tanh(att) @ v                                 # (t,)
    attn    = renorm(softmax(score) * mask)                 # (t,)
    c_t     = attn @ EO[b]                                  # (n,)
    cov_next= cov + attn

Data-parallel over batch across 8 NeuronCores (8 batches/core, params
replicated, no collectives).

HBM strategy: EF (n-major transposed) and EO (t-major) are cast to bf16 and
permuted on the host into partition-contiguous layouts, so each batch is ONE
2 MB HWDGE dma_start with 16 KB per-partition lines (near line rate). The
big stream owns the nc.sync ring in FIFO need-order: W_d(fp8), EF0, EO0,
EF1, ... Total ~33 MB/core -> ~92 us DMA floor.

Engine split (att is n-major: partition = n, free = t):
  - DVE:     att_pre = EFT + W_c[n] * cov_bcast   (scalar_tensor_tensor,
             bf16 2x mode; W_c as per-partition scalar column)
  - ScalarE: th = tanh(att_pre + dec[n])          (dec as per-partition bias)
  - PE:      score[t] = sum_n v[n]*th[n,t] as matmuls with lhsT = v column
             (accumulating [1,1024] PSUM row over the 8 n-tiles); c_t via
             lhsT = attn column over t-major EO tiles (interleaved with the
             next batch's score matmuls to keep the PE warm).
  - GpSimd:  all small DMAs (SWDGE) + the cross-partition softmax-sum
             (partition_all_reduce on [8,1]).
  - softmax runs on an [8,128] grid (one cheap exp) entirely off the
    ScalarE/DVE hot path; coverage/attn outputs stream out as [8,128].
dec_fea = s_t_hat @ W_d.T + b_d is a tiny fp8 PE matmul at init (W_d.T and
s_t_hat.T pre-cast fp8-e4m3 on host); cov broadcasts and dec columns are all
prebuilt during the initial DMA ramp so the steady state is pure
stream -> stt -> tanh -> matmul.
"""

import sys

if "/opt/trn_rl_repo" not in sys.path:
    sys.path.insert(0, "/opt/trn_rl_repo")

import ml_dtypes
import numpy as np

import concourse.bass as bass
import concourse.bass_isa as bass_isa
import concourse.mybir as mybir
import concourse.tile as tile
from concourse import bacc
from concourse.bass_utils import run_bass_kernel_spmd
from concourse.masks import make_identity

F32 = mybir.dt.float32
BF16 = mybir.dt.bfloat16
FP8 = mybir.dt.float8e4
AF = mybir.ActivationFunctionType
ALU = mybir.AluOpType

N_CORES = 8
B = 64
NB = B // N_CORES  # local batches per core
T = 1024
N = 1024
TJ = T // 128       # 128-tiles per batch (both t- and n- direction)
KT = N // 128       # k-tiles for the W_d matvec


def build_bass(nb: int = NB) -> bass.Bass:
    nc = bacc.Bacc()

    eft_d = nc.declare_dram_parameter("eft_bf16", [nb, 128, TJ, T], BF16, isOutput=False)
    eo_d = nc.declare_dram_parameter("eo_bf16", [nb, 128, TJ, N], BF16, isOutput=False)
    mk_d = nc.declare_dram_parameter("enc_padding_mask", [nb, T], F32, isOutput=False)
    cv_d = nc.declare_dram_parameter("coverage", [nb, T], F32, isOutput=False)
    wdt_d = nc.declare_dram_parameter("W_d_T", [N, N], FP8, isOutput=False)
    st_d = nc.declare_dram_parameter("s_t_hat_T", [N, nb], FP8, isOutput=False)
    bd_d = nc.declare_dram_parameter("b_d", [N], BF16, isOutput=False)
    wc_d = nc.declare_dram_parameter("W_c", [N], BF16, isOutput=False)
    v_d = nc.declare_dram_parameter("v", [N], BF16, isOutput=False)
    ct_o = nc.declare_dram_parameter("c_t", [nb, N], F32, isOutput=True)
    at_o = nc.declare_dram_parameter("attn", [nb, T], F32, isOutput=True)
    cn_o = nc.declare_dram_parameter("coverage_next", [nb, T], F32, isOutput=True)

    with tile.TileContext(nc) as tc:
        with (
            tc.tile_pool(name="consts", bufs=1) as consts,
            tc.tile_pool(name="wdtp", bufs=1) as wdtp,
            tc.tile_pool(name="covp", bufs=1) as covp,
            tc.tile_pool(name="efp", bufs=3) as efp,
            tc.tile_pool(name="eop", bufs=3) as eop,
            tc.tile_pool(name="attp", bufs=3) as attp,
            tc.tile_pool(name="thp", bufs=3) as thp,
            tc.tile_pool(name="smal", bufs=4) as smal,
            tc.tile_pool(name="rowstg", bufs=2) as rowstg,
            tc.tile_pool(name="psS", bufs=1, space="PSUM") as psS,
            tc.tile_pool(name="psC", bufs=1, space="PSUM") as psC,
            tc.tile_pool(name="psT", bufs=2, space="PSUM") as psT,
        ):
            # ---------------- big-stream DMAs (sync ring, FIFO need-order) ----
            wdt_all = wdtp.tile([128, KT, N], FP8)
            nc.sync.dma_start(
                out=wdt_all, in_=wdt_d.rearrange("(kj p) n -> p kj n", p=128)
            )
            ef_bufs = {}
            eo_bufs = {}

            def load_ef(b, split=1):
                t = efp.tile([128, TJ, T], BF16, tag="ef")
                step = TJ // split
                for s in range(split):
                    nc.sync.dma_start(
                        out=t[:, s * step:(s + 1) * step, :],
                        in_=eft_d[b, :, s * step:(s + 1) * step, :],
                    )
                ef_bufs[b] = t

            def load_eo(b):
                t = eop.tile([128, TJ, N], BF16, tag="eo")
                nc.sync.dma_start(out=t, in_=eo_d[b])
                eo_bufs[b] = t

            load_ef(0, split=2)
            load_eo(0)
            load_ef(1)
            load_eo(1)

            # ---------------- constants / small inputs (gpsimd ring) ----------
            ident = consts.tile([128, 128], F32)
            make_identity(nc, ident)
            ident_b = consts.tile([128, 128], BF16)
            nc.vector.tensor_copy(ident_b, ident)
            ones_f32 = consts.tile([1, T], F32)
            nc.vector.memset(ones_f32, 1.0)
            ones_b = consts.tile([1, T], BF16)
            nc.vector.tensor_copy(ones_b, ones_f32)

            sT_all = consts.tile([128, KT, 32], FP8)
            nc.gpsimd.dma_start(
                out=sT_all[:, :, 0:nb],
                in_=st_d.rearrange("(kj p) b -> p kj b", p=128),
            )
            bd_b = consts.tile([1, N], BF16)
            nc.gpsimd.dma_start(out=bd_b, in_=bd_d[None, :])
            v8 = consts.tile([TJ, 128], BF16)
            nc.gpsimd.dma_start(out=v8, in_=v_d.rearrange("(j t) -> j t", j=TJ))
            wc8 = consts.tile([TJ, 128], BF16)
            nc.gpsimd.dma_start(out=wc8, in_=wc_d.rearrange("(j t) -> j t", j=TJ))
            covrow_all = consts.tile([1, nb, T], BF16)
            nc.gpsimd.dma_start(out=covrow_all, in_=cv_d[None, :, :])  # f32->bf16 cast
            mk8_all = consts.tile([TJ, nb, 128], F32)
            nc.gpsimd.dma_start(
                out=mk8_all, in_=mk_d.rearrange("b (j t) -> j b t", j=TJ)
            )
            cov8_all = consts.tile([TJ, nb, 128], F32)
            nc.gpsimd.dma_start(
                out=cov8_all, in_=cv_d.rearrange("b (j t) -> j b t", j=TJ)
            )
            ones8 = consts.tile([TJ, 1], F32)
            nc.vector.memset(ones8, 1.0)

            # v / W_c as per-partition columns: transpose [8,128] -> [128,8]
            v_cols = consts.tile([128, TJ], BF16)
            wc_cols = consts.tile([128, TJ], BF16)
            for src, dst in ((v8, v_cols), (wc8, wc_cols)):
                pst = psT.tile([128, TJ], BF16, tag="tscratchb")
                nc.tensor.matmul(
                    pst, lhsT=src, rhs=ident_b[0:TJ, 0:TJ], is_transpose=True,
                    start=True, stop=True,
                )
                nc.vector.tensor_copy(dst, pst)

            # dec_fea = s_t_hat @ W_d.T + b_d  (fp8 matmuls, tiny), then
            # transpose into per-partition columns dec_cols[:, j, b]
            dec_cols = consts.tile([128, TJ, nb], F32)
            for h in range(2):
                sl = slice(h * 512, (h + 1) * 512)
                psd = psT.tile([nb, 512], F32, tag="tscratch")
                for kj in range(KT):
                    nc.tensor.matmul(
                        psd, lhsT=sT_all[:, kj, 0:nb], rhs=wdt_all[:, kj, sl],
                        start=(kj == 0), stop=False,
                    )
                nc.tensor.matmul(
                    psd, lhsT=ones_b[0:1, 0:nb], rhs=bd_b[0:1, sl],
                    start=False, stop=True,
                )
                dec8 = smal.tile([nb, 512], F32, tag="dec8")
                nc.vector.tensor_copy(dec8, psd)
                for jj in range(4):
                    j = h * 4 + jj
                    pst = psT.tile([128, TJ], F32, tag="tscratch")
                    nc.tensor.matmul(
                        pst[:, 0:nb], lhsT=dec8[:, jj * 128:(jj + 1) * 128],
                        rhs=ident[0:nb, 0:nb], is_transpose=True,
                        start=True, stop=True,
                    )
                    nc.vector.tensor_copy(dec_cols[:, j, :], pst[:, 0:nb])

            # cov broadcast tiles for every batch (built during the DMA ramp)
            covb_all = covp.tile([128, nb, T], BF16)
            for b in range(nb):
                for h in range(2):
                    cps = psT.tile([128, 512], F32, tag="tscratch")
                    nc.tensor.matmul(
                        cps, lhsT=ones_b[0:1, 0:128],
                        rhs=covrow_all[0:1, b, h * 512:(h + 1) * 512],
                        start=True, stop=True,
                    )
                    nc.vector.tensor_copy(
                        covb_all[:, b, h * 512:(h + 1) * 512], cps
                    )

            # ---------------- main loop over local batches ----------------
            attn_cols = {}
            ct_ps = {}

            def softmax_block(b, ps_s):
                score_row = rowstg.tile([1, T], F32, tag="score")
                nc.vector.tensor_copy(score_row, ps_s)
                score8 = smal.tile([TJ, 128], F32, tag="s8")
                nc.gpsimd.dma_start(
                    out=score8,
                    in_=score_row[0:1, :].rearrange("p (j t) -> p j t", j=TJ),
                )
                e8 = smal.tile([TJ, 128], F32, tag="e8")
                # scores are O(1) (|s| < ~3): plain exp is safe, skip max-sub
                nc.scalar.activation(e8, score8, AF.Exp)
                e8m = smal.tile([TJ, 128], F32, tag="e8m")
                msum8 = smal.tile([TJ, 1], F32, tag="msum")
                nc.vector.scalar_tensor_tensor(
                    out=e8m, in0=e8, scalar=1.0, in1=mk8_all[:, b, :],
                    op0=ALU.mult, op1=ALU.mult, accum_out=msum8,
                )
                rsum8 = smal.tile([TJ, 1], F32, tag="rsum")
                nc.gpsimd.partition_all_reduce(
                    rsum8, msum8, channels=TJ, reduce_op=bass_isa.ReduceOp.add
                )
                rs8 = smal.tile([TJ, 1], F32, tag="rs")
                nc.vector.reciprocal(rs8, rsum8)
                attn8 = smal.tile([TJ, 128], F32, tag="attn8")
                nc.vector.tensor_scalar_mul(attn8, e8m, rs8)
                covn8 = smal.tile([TJ, 128], F32, tag="covn8")
                nc.vector.tensor_add(covn8, cov8_all[:, b, :], attn8)
                nc.gpsimd.dma_start(
                    out=at_o[b].rearrange("(j t) -> j t", j=TJ), in_=attn8
                )
                nc.gpsimd.dma_start(
                    out=cn_o[b].rearrange("(j t) -> j t", j=TJ), in_=covn8
                )
                acp = psT.tile([128, TJ], F32, tag="tscratch")
                nc.tensor.matmul(
                    acp, lhsT=attn8, rhs=ident[0:TJ, 0:TJ], is_transpose=True,
                    start=True, stop=True,
                )
                acols = smal.tile([128, TJ], BF16, tag="acols")
                nc.vector.tensor_copy(acols, acp)
                attn_cols[b] = acols

            def finish_ct(b):
                ps_c = ct_ps.pop(b)
                eo_bufs.pop(b)
                ct_row = rowstg.tile([1, N], F32, tag="ctb")
                nc.vector.tensor_copy(ct_row, ps_c)
                nc.gpsimd.dma_start(out=ct_o[None, b, :], in_=ct_row)

            for b in range(nb):
                eft = ef_bufs.pop(b)
                if b + 2 < nb:
                    load_ef(b + 2)
                    load_eo(b + 2)
                ps_s = psS.tile([1, T], F32, tag="score_ps")
                if b > 0:
                    ct_ps[b - 1] = psC.tile([1, N], F32, tag="ct_psum", name=f"ctps{b}")
                for j in range(TJ):
                    att_pre = attp.tile([128, T], BF16, tag="attp")
                    nc.vector.scalar_tensor_tensor(
                        out=att_pre, in0=covb_all[:, b, :],
                        scalar=wc_cols[:, j:j + 1], in1=eft[:, j, :],
                        op0=ALU.mult, op1=ALU.add,
                    )
                    th = thp.tile([128, T], BF16, tag="th")
                    nc.scalar.activation(
                        th, att_pre, AF.Tanh, bias=dec_cols[:, j, b:b + 1]
                    )
                    for h in range(2):
                        nc.tensor.matmul(
                            ps_s[:, h * 512:(h + 1) * 512],
                            lhsT=v_cols[:, j:j + 1],
                            rhs=th[:, h * 512:(h + 1) * 512],
                            start=(j == 0), stop=(j == TJ - 1),
                            skip_group_check=True,
                        )
                    if b > 0:
                        acols = attn_cols[b - 1]
                        eot = eo_bufs[b - 1]
                        for h in range(2):
                            nc.tensor.matmul(
                                ct_ps[b - 1][:, h * 512:(h + 1) * 512],
                                lhsT=acols[:, j:j + 1],
                                rhs=eot[:, j, h * 512:(h + 1) * 512],
                                start=(j == 0), stop=(j == TJ - 1),
                                skip_group_check=True,
                            )
                softmax_block(b, ps_s)
                if b > 0:
                    finish_ct(b - 1)

            # trailing c_t for the last batch
            b = nb - 1
            ct_ps[b] = psC.tile([1, N], F32, tag="ct_psum", name="ctps_last")
            acols = attn_cols[b]
            eot = eo_bufs[b]
            for j in range(TJ):
                for h in range(2):
                    nc.tensor.matmul(
                        ct_ps[b][:, h * 512:(h + 1) * 512],
                        lhsT=acols[:, j:j + 1],
                        rhs=eot[:, j, h * 512:(h + 1) * 512],
                        start=(j == 0), stop=(j == TJ - 1),
                        skip_group_check=True,
                    )
            finish_ct(b)

    nc.finalize()
    return nc


_CACHE: dict = {}


def _get_nc() -> bass.Bass:
    if "nc" not in _CACHE:
        _CACHE["nc"] = build_bass(NB)
    return _CACHE["nc"]


def make_in_maps(inputs: dict) -> list:
    f = lambda x: np.ascontiguousarray(np.asarray(x), dtype=np.float32)
    s = f(inputs["s_t_hat"])
    eo = f(inputs["encoder_outputs"])
    ef = f(inputs["encoder_feature"]).reshape(B, T, N)
    mk = f(inputs["enc_padding_mask"])
    cv = f(inputs["coverage"])
    fp8 = ml_dtypes.float8_e4m3fn
    bf = ml_dtypes.bfloat16
    wdt = np.ascontiguousarray(f(inputs["W_d"]).T).astype(fp8)
    bd = f(inputs["b_d"])
    wc = f(inputs["W_c"])
    vv = f(inputs["v"])
    # EF n-major: [b, p, j, t] with n = 128*j + p   (16KB partition lines)
    ef_b = np.ascontiguousarray(
        ef.astype(bf).reshape(B, T, TJ, 128).transpose(0, 3, 2, 1)
    )
    # EO t-major: [b, p, j, n] with t = 128*j + p
    eo_b = np.ascontiguousarray(
        eo.astype(bf).reshape(B, TJ, 128, N).transpose(0, 2, 1, 3)
    )
    in_maps = []
    for i in range(N_CORES):
        sl = slice(i * NB, (i + 1) * NB)
        in_maps.append({
            "eft_bf16": ef_b[sl],
            "eo_bf16": eo_b[sl],
            "s_t_hat_T": np.ascontiguousarray(s[sl].T).astype(fp8),
            "enc_padding_mask": mk[sl],
            "coverage": cv[sl],
            "W_d_T": wdt,
            "b_d": bd.astype(bf),
            "W_c": wc.astype(bf),
            "v": vv.astype(bf),
        })
    return in_maps


def gather_outputs(results: list):
    c_t = np.concatenate([results[i]["c_t"] for i in range(N_CORES)], axis=0)
    attn = np.concatenate([results[i]["attn"] for i in range(N_CORES)], axis=0)
    covn = np.concatenate(
        [results[i]["coverage_next"] for i in range(N_CORES)], axis=0
    )
    return c_t, attn, covn


def kernel(**inputs):
    nc = _get_nc()
    in_maps = make_in_maps(inputs)
    res = run_bass_kernel_spmd(nc, in_maps, core_ids=list(range(N_CORES)))
    return gather_outputs(res.results)
